# revision 17
# baseline (speedup 1.0000x reference)
"""Trainium2 Bass kernel for GAT relation-to-entity message passing.

Contract: kernel(**inputs) takes the FULL unsharded inputs (x_e, x_r,
edge_index, rel, w_h, w_t, w_r) and returns the FULL [100000, 256] float32
output, distributing work over 8 NeuronCores internally.

Strategy (per core, no collectives): destination nodes are sharded 8 ways
(12500 per core); each core computes both the head- and tail-direction
aggregations for its node range. The host shards and permutes edges (the
"scatter-reduce" sharding from the problem hint): per (direction,
rel-block-of-128, 512-node PSUM tile) = "cell", edges are sorted by
destination node and greedily packed into a fixed number of 128-edge
chunks, where chunk k may only hold destinations in a fixed overlapping
column range [s_k, e_k) (~105 of the tile's 512 columns). The chunk count
and ranges depend only on expected cell occupancy (Poisson bounds), so the
instruction stream is input-independent and shared by all 8 cores; a
feasibility check falls back to a roomier structure for pathological
inputs.

Per chunk the device builds a rel one-hot fused with the softmax numerator
ex (one DVE tensor_scalar: is_equal then mult, fp16 4x mode) and a narrow
node one-hot [128, w] (second tensor_scalar), then one fp16 TensorE matmul
accumulates W[rel, node] into the PSUM bank at static column offset; chunk
ranges tile the bank so untouched columns get written as zeros. Per (tile,
block) ScalarE copies W to SBUF fp16; per 128-node subtile 8 matmuls
compute W^T @ [x_r | 1] -> [node, 129]; the result is normalized by the
denominator (+1e-16, matching the reference) and DMA'd out.

ex = exp(u) with u = lrelu(z) - Cz, z = s_dst[dst] + s_r[rel] built from
the replicated score vectors s_* = x @ w_*, and Cz the per-destination
segment max of lrelu(z) - exactly the reference's numerically-stable
softmax. u is computed on host (same host/device split as the accepted
baseline, which computed z and Cz on host); exp, scatter, aggregation and
normalization all run on device.
"""

import sys
import numpy as np

for _p in ("/opt/trn_rl_repo", "/root/.axon_site/_ro/trn_rl_repo",
           "/opt/pypackages", "/root/.axon_site/_ro/pypackages"):
    if _p not in sys.path:
        sys.path.append(_p)

import concourse.bass as bass
import concourse.tile as tile
from concourse import bacc, mybir
from concourse.bass_utils import run_bass_kernel_spmd
from contextlib import ExitStack

F32 = mybir.dt.float32
F16 = mybir.dt.float16
P = 128
N_CORES = 8
N_NODES = 100000
NPC = N_NODES // N_CORES            # 12500 nodes per core
TILE = 512                          # PSUM bank width (fp32) = node tile
N_TILES = (NPC + TILE - 1) // TILE  # 25, last ragged (212)
N_BLK = 8                           # rel blocks of 128 (cover 1000 rels)
N_REL = 1000
E_TOT = 1600000
U_PAD = -30000.0                    # exp(U_PAD) == 0 in fp32
IOREP = 16                          # j-repeat factor of the shared iota

_module_cache = {}


def _tw(t):
    return min(TILE, NPC - t * TILE)


def _n_rel_blk(bk):
    return min(128, N_REL - bk * 128)


def _cell_lambda(t, bk):
    # expected edges per (core, dir, tile, block)
    return (E_TOT / N_CORES) * (_n_rel_blk(bk) / N_REL) * (_tw(t) / NPC)


WIN = 128                           # fixed node one-hot window width


def _structure(level):
    """Input-independent chunk structure. Returns list over (d, t, bk) of
    list of window starts s (window = [s, s+WIN)), in emission order."""
    extra = [0, 1, 2, 4][level]
    cells = []
    for d in range(2):
        for t in range(N_TILES):
            tw = _tw(t)
            for bk in range(N_BLK):
                lam = _cell_lambda(t, bk)
                k = int(np.ceil((lam + 4.0 * np.sqrt(lam)) / P)) + extra
                k = max(k, 2)
                span = tw - WIN
                starts = []
                for j in range(k):
                    s = int(round(j * span / (k - 1))) if k > 1 else 0
                    starts.append((s // 2) * 2)
                cells.append(tuple(starts))
    return tuple(cells)


class _Infeasible(Exception):
    pass


def _host_prep(x_e, x_r, edge_index, rel, w_h, w_t, w_r, level):
    """Returns (in_maps, meta). Raises _Infeasible if the level's structure
    cannot hold this input."""
    x_e = np.asarray(x_e, np.float32)
    x_r = np.asarray(x_r, np.float32)
    ei = np.asarray(edge_index).astype(np.int64)
    rl_all = np.asarray(rel).astype(np.int64)
    w_h = np.asarray(w_h, np.float32)
    w_t = np.asarray(w_t, np.float32)
    w_r = np.asarray(w_r, np.float32)

    s_h = x_e @ w_h
    s_t = x_e @ w_t
    s_r = x_r @ w_r

    cells = _structure(level)
    n_chunks = sum(len(r) for r in cells)
    C = n_chunks

    # io: [0:128) plain iota (rel one-hot in0); [128:] = iota repeated
    # IOREP times (node one-hot TT in1, strided view).
    io_np = np.zeros((P, 128 + 128 * IOREP), np.float16)
    io_np[:, 0:128] = np.arange(128, dtype=np.float16)[None, :]
    io_np[:, 128:] = np.repeat(np.arange(128, dtype=np.float16),
                               IOREP)[None, :]
    xr_np = np.zeros((N_BLK, P, 129), np.float32)
    nr = x_r.shape[0]
    for b in range(N_BLK):
        take = min(P, max(0, nr - b * P))
        if take > 0:
            xr_np[b, :take, 0:128] = x_r[b * P:b * P + take]
        xr_np[b, :, 128] = 1.0
    xr_np = xr_np.astype(np.float16)

    # chunk column index for (d, t, bk, j): precompute offsets. nl gets its
    # own column map with even per-cell starts (DVE 2x mode needs 4B-aligned
    # operands; fp16 columns at odd indices are only 2B-aligned).
    cell_off = np.zeros(len(cells) + 1, np.int64)
    nl_off = np.zeros(len(cells), np.int64)
    nc2 = 0
    for i, r in enumerate(cells):
        cell_off[i + 1] = cell_off[i] + len(r)
        nl_off[i] = nc2
        nc2 += len(r) + (len(r) & 1)
    C2 = nc2

    in_maps = []
    for c in range(N_CORES):
        u_arr = np.full((P, C), U_PAD, np.float32)
        r_arr = np.zeros((P, C), np.float32)
        n_arr = np.zeros((P, C2), np.float16)  # window-local node coords
        in_maps.append({"u": u_arr, "rl": r_arr, "nl": n_arr,
                        "xr": xr_np, "io": io_np})

    for d, (dst_all, s_dst) in enumerate(((ei[0], s_h), (ei[1], s_t))):
        z = (s_dst[dst_all] + s_r[rl_all]).astype(np.float32)
        lr = np.where(z >= 0, z, 0.01 * z).astype(np.float32)
        order = np.argsort(dst_all, kind="stable")
        ds, ls = dst_all[order], lr[order]
        m = np.full(N_NODES, -np.inf, np.float32)
        uniq, starts = np.unique(ds, return_index=True)
        m[uniq] = np.maximum.reduceat(ls, starts)
        u_all = (lr - m[dst_all]).astype(np.float32)

        core_of = dst_all // NPC
        for c in range(N_CORES):
            msk = core_of == c
            dl = (dst_all[msk] - c * NPC).astype(np.int64)
            r = rl_all[msk]
            u = u_all[msk]
            t = dl >> 9
            blk = r >> 7
            o = np.argsort((blk * N_TILES + t) * (NPC + 1) + dl,
                           kind="stable")
            dl, r, u = dl[o], r[o], u[o]
            seg = blk[o] * N_TILES + t[o]
            bounds = np.nonzero(np.diff(seg))[0] + 1
            seg_starts = np.concatenate([[0], bounds, [len(dl)]])
            ua, ra, na = (in_maps[c]["u"], in_maps[c]["rl"],
                          in_maps[c]["nl"])
            for i in range(len(seg_starts) - 1):
                a, b = int(seg_starts[i]), int(seg_starts[i + 1])
                if a == b:
                    continue
                tt, bk = int(dl[a] >> 9), int(r[a] >> 7)
                cid = (d * N_TILES + tt) * N_BLK + bk
                starts = cells[cid]
                nl = dl[a:b] & 511
                rr = (r[a:b] & 127).astype(np.float32)
                uu = u[a:b]
                n = b - a
                p = 0
                for j, s in enumerate(starts):
                    if p >= n:
                        break
                    if nl[p] < s:
                        raise _Infeasible(f"low d{d} t{tt} b{bk} j{j}")
                    hi = int(np.searchsorted(nl, s + WIN, side="left"))
                    take = min(P, hi - p)
                    if take <= 0:
                        continue
                    ci = cell_off[cid] + j
                    ua[:take, ci] = uu[p:p + take]
                    ra[:take, ci] = rr[p:p + take]
                    na[:take, nl_off[cid] + j] = \
                        (nl[p:p + take] - s).astype(np.float16)
                    p += take
                if p < n:
                    raise _Infeasible(f"cap d{d} t{tt} b{bk}")
    meta = (level, C, C2)
    return in_maps, meta


def _build_module(meta, repeat=1):
    level, C, C2 = meta
    cells = _structure(level)
    nc = bacc.Bacc("TRN2", target_bir_lowering=False, debug=False,
                   num_devices=N_CORES)

    u_ap = nc.dram_tensor("u", [P, C], F32, kind="ExternalInput").ap()
    rl_ap = nc.dram_tensor("rl", [P, C], F32, kind="ExternalInput").ap()
    nl_ap = nc.dram_tensor("nl", [P, C2], F16,
                           kind="ExternalInput").ap()
    xr_ap = nc.dram_tensor("xr", [N_BLK, P, 129], F16,
                           kind="ExternalInput").ap()
    io_ap = nc.dram_tensor("io", [P, 128 + 128 * IOREP], F16,
                           kind="ExternalInput").ap()
    yh_ap = nc.dram_tensor("yh", [NPC, 128], F32, kind="ExternalOutput").ap()
    yt_ap = nc.dram_tensor("yt", [NPC, 128], F32, kind="ExternalOutput").ap()
    y_aps = [yh_ap, yt_ap]

    with tile.TileContext(nc) as tc, ExitStack() as ctx:
        big = ctx.enter_context(tc.tile_pool(name="big", bufs=1))
        work = ctx.enter_context(tc.tile_pool(name="work", bufs=16))
        wrk2 = ctx.enter_context(tc.tile_pool(name="wrk2", bufs=4))
        wsbp = ctx.enter_context(tc.tile_pool(name="wsbp", bufs=18))
        outp = ctx.enter_context(tc.tile_pool(name="outp", bufs=4))
        psw = ctx.enter_context(tc.tile_pool(name="psw", bufs=4,
                                             space="PSUM"))
        pso = ctx.enter_context(tc.tile_pool(name="pso", bufs=2,
                                             space="PSUM"))

        ut = big.tile([P, C], F32, tag="ut")
        rlt = big.tile([P, C], F32, tag="rlt")
        nlt = big.tile([P, C2], F16, tag="nlt")
        ext = big.tile([P, C], F32, tag="ext")
        iot = big.tile([P, 128 + 128 * IOREP], F16, tag="iot")
        xrt = big.tile([P, N_BLK * 129], F16, tag="xrt")

        nc.sync.dma_start(ut[:], u_ap[:])
        nc.sync.dma_start(rlt[:], rl_ap[:])
        nc.sync.dma_start(nlt[:], nl_ap[:])
        nc.sync.dma_start(iot[:], io_ap[:])
        for b in range(N_BLK):
            nc.sync.dma_start(xrt[:, b * 129:(b + 1) * 129], xr_ap[b])

        NSL = 8
        sl = (C + NSL - 1) // NSL
        for i in range(NSL):
            s0, s1 = i * sl, min((i + 1) * sl, C)
            if s0 >= s1:
                continue
            nc.scalar.activation(ext[:, s0:s1], ut[:, s0:s1],
                                 mybir.ActivationFunctionType.Exp)

        for _rep in range(repeat):
            ci = 0
            ci2 = 0
            mi = 0
            for d in range(2):
                for t in range(N_TILES):
                    tw = _tw(t)
                    wsb_tiles = []
                    for bk in range(N_BLK):
                        starts = cells[mi]
                        mi += 1
                        pw = psw.tile([P, TILE], F32, space="PSUM", tag="pw")
                        n_ch = len(starts)
                        # node one-hots for the whole cell, one interleaved
                        # TT: out[p, j, c] = (io_rep[j] == nl[p, ci+c])
                        ohna = wrk2.tile([P, 128 * IOREP], F16, tag="ohna")
                        ohna_v = ohna[:, 0:128 * n_ch].rearrange(
                            "p (j c) -> p j c", c=n_ch)
                        nc.vector.tensor_tensor(
                            out=ohna_v,
                            in0=nlt[:, None, ci2:ci2 + n_ch].to_broadcast(
                                [P, 128, n_ch]),
                            in1=iot[:, 128:].rearrange(
                                "p (j c) -> p j c", c=IOREP)[:, :, 0:n_ch],
                            op=mybir.AluOpType.is_equal)
                        for k, s in enumerate(starts):
                            exr = work.tile([P, 128], F16, tag="exr")
                            nc.vector.tensor_scalar(
                                out=exr[:], in0=iot[:, 0:128],
                                scalar1=rlt[:, ci:ci + 1],
                                scalar2=ext[:, ci:ci + 1],
                                op0=mybir.AluOpType.is_equal,
                                op1=mybir.AluOpType.mult)
                            nc.tensor.matmul(pw[:, s:s + WIN], lhsT=exr[:],
                                             rhs=ohna_v[:, :, k],
                                             start=(k == 0),
                                             stop=(k == n_ch - 1))
                            ci += 1
                        ci2 += n_ch + (n_ch & 1)
                        wsb = wsbp.tile([P, TILE], F16, tag="wsb")
                        nc.scalar.activation(
                            wsb[:, 0:tw], pw[:, 0:tw],
                            mybir.ActivationFunctionType.Copy)
                        wsb_tiles.append(wsb)
                    for sub0 in range(0, tw, P):
                        sw = min(P, tw - sub0)
                        po = pso.tile([P, TILE], F32, space="PSUM", tag="po")
                        for bk in range(N_BLK):
                            nc.tensor.matmul(
                                po[:sw, 0:129],
                                lhsT=wsb_tiles[bk][:, sub0:sub0 + sw],
                                rhs=xrt[:, bk * 129:(bk + 1) * 129],
                                start=(bk == 0), stop=(bk == N_BLK - 1))
                        den = outp.tile([P, 1], F32, tag="den")
                        nc.vector.tensor_scalar(
                            out=den[:sw], in0=po[:sw, 128:129],
                            scalar1=1e-16, scalar2=None,
                            op0=mybir.AluOpType.add)
                        nc.vector.reciprocal(out=den[:sw], in_=den[:sw])
                        ob = outp.tile([P, 128], F32, tag="ob")
                        nc.scalar.activation(
                            ob[:sw], po[:sw, 0:128],
                            mybir.ActivationFunctionType.Copy,
                            scale=den[:sw])
                        node0 = t * TILE + sub0
                        nc.sync.dma_start(
                            y_aps[d][node0:node0 + sw, :], ob[:sw, :])
    nc.compile()
    return nc


def _get_module(meta, repeat=1):
    key = (meta, repeat)
    if key not in _module_cache:
        _module_cache[key] = _build_module(meta, repeat)
    return _module_cache[key]


def kernel(x_e, x_r, edge_index, rel, w_h, w_t, w_r):
    for level in range(4):
        try:
            in_maps, meta = _host_prep(x_e, x_r, edge_index, rel,
                                       w_h, w_t, w_r, level)
            break
        except _Infeasible:
            if level == 3:
                raise
    nc = _get_module(meta)
    res = run_bass_kernel_spmd(nc, in_maps, core_ids=list(range(N_CORES)))
    out = np.zeros((N_NODES, 256), np.float32)
    for c in range(N_CORES):
        out[c * NPC:(c + 1) * NPC, 0:128] = res.results[c]["yh"]
        out[c * NPC:(c + 1) * NPC, 128:256] = res.results[c]["yt"]
    return out


# revision 21
# speedup vs baseline: 1.3209x; 1.3209x over previous
"""Trainium2 Bass kernel for GAT relation-to-entity message passing.

Contract: kernel(**inputs) takes the FULL unsharded inputs (x_e, x_r,
edge_index, rel, w_h, w_t, w_r) and returns the FULL [100000, 256] float32
output, distributing work over 8 NeuronCores internally.

Strategy (per core, no collectives): destination nodes are sharded 8 ways
(12500 per core); each core computes both the head- and tail-direction
aggregations for its node range. The host shards and permutes edges (the
"scatter-reduce" sharding from the problem hint): per (direction,
rel-block-of-128, 512-node PSUM tile) = "cell", edges are sorted by
destination node and greedily packed into a fixed number of 128-edge
chunks, where chunk k may only hold destinations in a fixed overlapping
column range [s_k, e_k) (~105 of the tile's 512 columns). The chunk count
and ranges depend only on expected cell occupancy (Poisson bounds), so the
instruction stream is input-independent and shared by all 8 cores; a
feasibility check falls back to a roomier structure for pathological
inputs.

Per chunk the device builds a rel one-hot fused with the softmax numerator
ex (one DVE tensor_scalar: is_equal then mult, fp16 4x mode) and a narrow
node one-hot [128, w] (second tensor_scalar), then one fp16 TensorE matmul
accumulates W[rel, node] into the PSUM bank at static column offset; chunk
ranges tile the bank so untouched columns get written as zeros. Per (tile,
block) ScalarE copies W to SBUF fp16; per 128-node subtile 8 matmuls
compute W^T @ [x_r | 1] -> [node, 129]; the result is normalized by the
denominator (+1e-16, matching the reference) and DMA'd out.

ex = exp(u) with u = lrelu(z) - Cz, z = s_dst[dst] + s_r[rel] built from
the replicated score vectors s_* = x @ w_*, and Cz the per-destination
segment max of lrelu(z) - exactly the reference's numerically-stable
softmax. u is computed on host (same host/device split as the accepted
baseline, which computed z and Cz on host); exp, scatter, aggregation and
normalization all run on device.
"""

import sys
import numpy as np

for _p in ("/opt/trn_rl_repo", "/root/.axon_site/_ro/trn_rl_repo",
           "/opt/pypackages", "/root/.axon_site/_ro/pypackages"):
    if _p not in sys.path:
        sys.path.append(_p)

import concourse.bass as bass
import concourse.tile as tile
from concourse import bacc, mybir
from concourse.bass_utils import run_bass_kernel_spmd
from contextlib import ExitStack

F32 = mybir.dt.float32
F16 = mybir.dt.float16
P = 128
N_CORES = 8
N_NODES = 100000
NPC = N_NODES // N_CORES            # 12500 nodes per core
TILE = 512                          # PSUM bank width (fp32) = node tile
N_TILES = (NPC + TILE - 1) // TILE  # 25, last ragged (212)
N_BLK = 8                           # rel blocks of 128 (cover 1000 rels)
N_REL = 1000
E_TOT = 1600000
U_PAD = -30000.0                    # exp(U_PAD) == 0 in fp32
IOREP = 16                          # j-repeat factor of the shared iota

_module_cache = {}


def _tw(t):
    return min(TILE, NPC - t * TILE)


def _n_rel_blk(bk):
    return min(128, N_REL - bk * 128)


def _cell_lambda(t, bk):
    # expected edges per (core, dir, tile, block)
    return (E_TOT / N_CORES) * (_n_rel_blk(bk) / N_REL) * (_tw(t) / NPC)


WIN = 128                           # fixed node one-hot window width


def _structure(level):
    """Input-independent chunk structure. Returns list over (d, t, bk) of
    list of window starts s (window = [s, s+WIN)), in emission order."""
    extra = [0, 1, 2, 4][level]
    cells = []
    for d in range(2):
        for t in range(N_TILES):
            tw = _tw(t)
            for bk in range(N_BLK):
                lam = _cell_lambda(t, bk)
                k = int(np.ceil((lam + 4.0 * np.sqrt(lam)) / P)) + extra
                k = max(k, 2)
                span = tw - WIN
                starts = []
                for j in range(k):
                    s = int(round(j * span / (k - 1))) if k > 1 else 0
                    starts.append((s // 2) * 2)
                cells.append(tuple(starts))
    return tuple(cells)


class _Infeasible(Exception):
    pass


def _host_prep(x_e, x_r, edge_index, rel, w_h, w_t, w_r, level):
    """Returns (in_maps, meta). Raises _Infeasible if the level's structure
    cannot hold this input."""
    x_e = np.asarray(x_e, np.float32)
    x_r = np.asarray(x_r, np.float32)
    ei = np.asarray(edge_index).astype(np.int64)
    rl_all = np.asarray(rel).astype(np.int64)
    w_h = np.asarray(w_h, np.float32)
    w_t = np.asarray(w_t, np.float32)
    w_r = np.asarray(w_r, np.float32)

    s_h = x_e @ w_h
    s_t = x_e @ w_t
    s_r = x_r @ w_r

    cells = _structure(level)
    n_chunks = sum(len(r) for r in cells)
    C = n_chunks

    # io: [0:128) plain iota (rel one-hot in0); [128:] = iota repeated
    # IOREP times (node one-hot TT in1, strided view).
    io_np = np.zeros((P, 128 + 128 * IOREP), np.float16)
    io_np[:, 0:128] = np.arange(128, dtype=np.float16)[None, :]
    io_np[:, 128:] = np.repeat(np.arange(128, dtype=np.float16),
                               IOREP)[None, :]
    xr_np = np.zeros((N_BLK, P, 129), np.float32)
    nr = x_r.shape[0]
    for b in range(N_BLK):
        take = min(P, max(0, nr - b * P))
        if take > 0:
            xr_np[b, :take, 0:128] = x_r[b * P:b * P + take]
        xr_np[b, :, 128] = 1.0
    xr_np = xr_np.astype(np.float16)

    # chunk column index for (d, t, bk, j): precompute offsets. nl gets its
    # own column map with even per-cell starts (DVE 2x mode needs 4B-aligned
    # operands; fp16 columns at odd indices are only 2B-aligned).
    cell_off = np.zeros(len(cells) + 1, np.int64)
    for i, r in enumerate(cells):
        cell_off[i + 1] = cell_off[i] + len(r)
    nl_off = cell_off[:-1]
    C2 = C

    in_maps = []
    for c in range(N_CORES):
        u_arr = np.full((P, C), U_PAD, np.float32)
        r_arr = np.zeros((P, C), np.float32)
        n_arr = np.zeros((P, C2), np.float16)  # window-local node coords
        in_maps.append({"u": u_arr, "rl": r_arr, "nl": n_arr,
                        "xr": xr_np, "io": io_np})

    for d, (dst_all, s_dst) in enumerate(((ei[0], s_h), (ei[1], s_t))):
        z = (s_dst[dst_all] + s_r[rl_all]).astype(np.float32)
        lr = np.where(z >= 0, z, 0.01 * z).astype(np.float32)
        order = np.argsort(dst_all, kind="stable")
        ds, ls = dst_all[order], lr[order]
        m = np.full(N_NODES, -np.inf, np.float32)
        uniq, starts = np.unique(ds, return_index=True)
        m[uniq] = np.maximum.reduceat(ls, starts)
        u_all = (lr - m[dst_all]).astype(np.float32)

        core_of = dst_all // NPC
        for c in range(N_CORES):
            msk = core_of == c
            dl = (dst_all[msk] - c * NPC).astype(np.int64)
            r = rl_all[msk]
            u = u_all[msk]
            t = dl >> 9
            blk = r >> 7
            o = np.argsort((blk * N_TILES + t) * (NPC + 1) + dl,
                           kind="stable")
            dl, r, u = dl[o], r[o], u[o]
            seg = blk[o] * N_TILES + t[o]
            bounds = np.nonzero(np.diff(seg))[0] + 1
            seg_starts = np.concatenate([[0], bounds, [len(dl)]])
            ua, ra, na = (in_maps[c]["u"], in_maps[c]["rl"],
                          in_maps[c]["nl"])
            for i in range(len(seg_starts) - 1):
                a, b = int(seg_starts[i]), int(seg_starts[i + 1])
                if a == b:
                    continue
                tt, bk = int(dl[a] >> 9), int(r[a] >> 7)
                cid = (d * N_TILES + tt) * N_BLK + bk
                starts = cells[cid]
                nl = dl[a:b] & 511
                rr = (r[a:b] & 127).astype(np.float32)
                uu = u[a:b]
                n = b - a
                p = 0
                for j, s in enumerate(starts):
                    if p >= n:
                        break
                    if nl[p] < s:
                        raise _Infeasible(f"low d{d} t{tt} b{bk} j{j}")
                    hi = int(np.searchsorted(nl, s + WIN, side="left"))
                    take = min(P, hi - p)
                    if take <= 0:
                        continue
                    ci = cell_off[cid] + j
                    ua[:take, ci] = uu[p:p + take]
                    ra[:take, ci] = rr[p:p + take]
                    na[:take, nl_off[cid] + j] = \
                        (nl[p:p + take] - s).astype(np.float16)
                    p += take
                if p < n:
                    raise _Infeasible(f"cap d{d} t{tt} b{bk}")
    meta = (level, C, C2)
    return in_maps, meta


def _build_module(meta, repeat=1):
    level, C, C2 = meta
    cells = _structure(level)
    nc = bacc.Bacc("TRN2", target_bir_lowering=False, debug=False,
                   num_devices=N_CORES)

    u_ap = nc.dram_tensor("u", [P, C], F32, kind="ExternalInput").ap()
    rl_ap = nc.dram_tensor("rl", [P, C], F32, kind="ExternalInput").ap()
    nl_ap = nc.dram_tensor("nl", [P, C2], F16,
                           kind="ExternalInput").ap()
    xr_ap = nc.dram_tensor("xr", [N_BLK, P, 129], F16,
                           kind="ExternalInput").ap()
    io_ap = nc.dram_tensor("io", [P, 128 + 128 * IOREP], F16,
                           kind="ExternalInput").ap()
    yh_ap = nc.dram_tensor("yh", [NPC, 128], F32, kind="ExternalOutput").ap()
    yt_ap = nc.dram_tensor("yt", [NPC, 128], F32, kind="ExternalOutput").ap()
    y_aps = [yh_ap, yt_ap]

    with tile.TileContext(nc) as tc, ExitStack() as ctx:
        big = ctx.enter_context(tc.tile_pool(name="big", bufs=1))
        work = ctx.enter_context(tc.tile_pool(name="work", bufs=24))
        wrk2 = ctx.enter_context(tc.tile_pool(name="wrk2", bufs=6))
        wsbp = ctx.enter_context(tc.tile_pool(name="wsbp", bufs=22))
        outp = ctx.enter_context(tc.tile_pool(name="outp", bufs=4))
        psw = ctx.enter_context(tc.tile_pool(name="psw", bufs=4,
                                             space="PSUM"))
        pso = ctx.enter_context(tc.tile_pool(name="pso", bufs=2,
                                             space="PSUM"))

        ut = big.tile([P, C], F32, tag="ut")
        rlt = big.tile([P, C], F32, tag="rlt")
        nlt = big.tile([P, C2], F16, tag="nlt")
        ext = big.tile([P, C], F32, tag="ext")
        iot = big.tile([P, 128 + 128 * IOREP], F16, tag="iot")
        xrt = big.tile([P, N_BLK * 129], F16, tag="xrt")

        nc.sync.dma_start(ut[:], u_ap[:])
        nc.sync.dma_start(rlt[:], rl_ap[:])
        nc.sync.dma_start(nlt[:], nl_ap[:])
        nc.sync.dma_start(iot[:], io_ap[:])
        for b in range(N_BLK):
            nc.sync.dma_start(xrt[:, b * 129:(b + 1) * 129], xr_ap[b])

        NSL = 8
        sl = (C + NSL - 1) // NSL
        for i in range(NSL):
            s0, s1 = i * sl, min((i + 1) * sl, C)
            if s0 >= s1:
                continue
            nc.scalar.activation(ext[:, s0:s1], ut[:, s0:s1],
                                 mybir.ActivationFunctionType.Exp)

        def stage_b(d, t, tw, wsb_tiles):
            for sub0 in range(0, tw, P):
                sw = min(P, tw - sub0)
                po = pso.tile([P, TILE], F32, space="PSUM", tag="po")
                for bk in range(N_BLK):
                    nc.tensor.matmul(
                        po[:sw, 0:129],
                        lhsT=wsb_tiles[bk][:, sub0:sub0 + sw],
                        rhs=xrt[:, bk * 129:(bk + 1) * 129],
                        start=(bk == 0), stop=(bk == N_BLK - 1))
                den = outp.tile([P, 1], F32, tag="den")
                nc.vector.tensor_scalar(
                    out=den[:sw], in0=po[:sw, 128:129],
                    scalar1=1e-16, scalar2=None,
                    op0=mybir.AluOpType.add)
                nc.vector.reciprocal(out=den[:sw], in_=den[:sw])
                ob = outp.tile([P, 128], F32, tag="ob")
                nc.scalar.activation(
                    ob[:sw], po[:sw, 0:128],
                    mybir.ActivationFunctionType.Copy,
                    scale=den[:sw])
                node0 = t * TILE + sub0
                nc.sync.dma_start(
                    y_aps[d][node0:node0 + sw, :], ob[:sw, :])

        for _rep in range(repeat):
            ci = 0
            ci2 = 0
            mi = 0
            # stage B trails stage A by one tile so the PE always has
            # next-tile scatter matmuls while ScalarE produces W copies
            pending = None
            for d in range(2):
                for t in range(N_TILES):
                    tw = _tw(t)
                    wsb_tiles = []
                    for bk in range(N_BLK):
                        starts = cells[mi]
                        mi += 1
                        pw = psw.tile([P, TILE], F32, space="PSUM", tag="pw")
                        n_ch = len(starts)
                        # node one-hots for the whole cell, one interleaved
                        # TT: out[p, j, c] = (io_rep[j] == nl[p, ci+c])
                        ohna = wrk2.tile([P, 128 * IOREP], F16, tag="ohna")
                        ohna_v = ohna[:, 0:128 * n_ch].rearrange(
                            "p (j c) -> p j c", c=n_ch)
                        nc.vector.tensor_tensor(
                            out=ohna_v,
                            in0=nlt[:, None, ci2:ci2 + n_ch].to_broadcast(
                                [P, 128, n_ch]),
                            in1=iot[:, 128:].rearrange(
                                "p (j c) -> p j c", c=IOREP)[:, :, 0:n_ch],
                            op=mybir.AluOpType.is_equal)
                        for k, s in enumerate(starts):
                            exr = work.tile([P, 128], F16, tag="exr")
                            nc.vector.tensor_scalar(
                                out=exr[:], in0=iot[:, 0:128],
                                scalar1=rlt[:, ci:ci + 1],
                                scalar2=ext[:, ci:ci + 1],
                                op0=mybir.AluOpType.is_equal,
                                op1=mybir.AluOpType.mult)
                            nc.tensor.matmul(pw[:, s:s + WIN], lhsT=exr[:],
                                             rhs=ohna_v[:, :, k],
                                             start=(k == 0),
                                             stop=(k == n_ch - 1))
                            ci += 1
                        ci2 += n_ch
                        wsb = wsbp.tile([P, TILE], F16, tag="wsb")
                        nc.scalar.activation(
                            wsb[:, 0:tw], pw[:, 0:tw],
                            mybir.ActivationFunctionType.Copy)
                        wsb_tiles.append(wsb)
                    if pending is not None:
                        stage_b(*pending)
                    pending = (d, t, tw, wsb_tiles)
            if pending is not None:
                stage_b(*pending)
    nc.compile()
    return nc


def _get_module(meta, repeat=1):
    key = (meta, repeat)
    if key not in _module_cache:
        _module_cache[key] = _build_module(meta, repeat)
    return _module_cache[key]


def kernel(x_e, x_r, edge_index, rel, w_h, w_t, w_r):
    for level in range(4):
        try:
            in_maps, meta = _host_prep(x_e, x_r, edge_index, rel,
                                       w_h, w_t, w_r, level)
            break
        except _Infeasible:
            if level == 3:
                raise
    nc = _get_module(meta)
    res = run_bass_kernel_spmd(nc, in_maps, core_ids=list(range(N_CORES)))
    out = np.zeros((N_NODES, 256), np.float32)
    for c in range(N_CORES):
        out[c * NPC:(c + 1) * NPC, 0:128] = res.results[c]["yh"]
        out[c * NPC:(c + 1) * NPC, 128:256] = res.results[c]["yt"]
    return out
